# revision 83
# baseline (speedup 1.0000x reference)
"""Trainium2 Bass kernel for CausalMessagePassing (B=8, N=2048, D=256, H=4).

Strategy: data-parallel across 8 NeuronCores, one graph per core.
Per-core dataflow, column-major spine with row-major ctx:
  x -> x^T (PE transpose); q^T,k^T col-major f32r.
  scores^T[j,i] = k_h^T.T @ q_h^T per head; the two heads of a pair share
  one PSUM mega tile [128, 2*512] so ONE exp instruction covers both
  heads (halves ACT instruction count, the bottleneck engine). Causal
  mask applied on-chip by a DVE multiply with a precomputed bf16
  triangle mask on the diagonal 128-chunk only.
  ctx is computed ROW-major in bf16: ctx[q, (d|sum)] += e_chunk[k, q].T
  @ [v|1][k, *] (lhsT = e chunk [128,128], rhs = v+ones [128,65]) -> 65
  moving rows per causal tile instead of >=256: ~halves the PE cost of
  the ctx pass (bf16 avoids the narrow-f32r 4x penalty). PSUM start
  flags are per-2KB-bank (one start per bank; the bank-wide pending-
  zero covers sibling accumulator regions).
  Normalization: per-partition reciprocal [128,2] + tensor_scalar_mul
  fused into the PSUM eviction (no partition_broadcast); ctx rows are
  transposed back to column-major (4 PE transposes per it packed into
  one PSUM quad) -> ectx^T.
  The message projection is folded into the update GEMM: with
  Wfused = Wo @ Wu_m and bo_wu = bo @ Wu_m (computed on-device off the
  critical path), u^T = relu(Wu_x^T x^T + Wfused^T ectx^T + bo_wu + bu)
  -- no separate Wo pass or m staging. PE-transpose u^T -> u, DMA out.
  Work is emitted in snake-ordered (hpair, it) windows; projections and
  tails trickle into the attention windows, each window's last ctx step
  + norms + transposes are deferred past the next window's pipeline
  ramp, and the final it3 tail is split into 256-col halves overlapped
  with the last window's drain.
"""
import sys

sys.path.insert(0, "/opt/trn_rl_repo")

import numpy as np

import concourse.bass as bass  # noqa: F401
import concourse.mybir as mybir
import concourse.tile as tile
from concourse import bacc
from concourse.masks import make_identity

B, N, DM, H = 8, 2048, 256, 4
HD = DM // H  # 64
NB = N // 128  # 16 j-blocks
IT = N // 512  # 4 i-tiles
F32 = mybir.dt.float32
F32R = mybir.dt.float32r
BF16 = mybir.dt.bfloat16


def build_program():
    nc = bacc.Bacc("TRN2", target_bir_lowering=False, debug=False)
    x_d = nc.dram_tensor("x", [N, DM], F32, kind="ExternalInput").ap()
    wq_d = nc.dram_tensor("wq", [DM, DM], F32, kind="ExternalInput").ap()
    wk_d = nc.dram_tensor("wk", [DM, DM], F32, kind="ExternalInput").ap()
    wv_d = nc.dram_tensor("wv", [DM, DM], F32, kind="ExternalInput").ap()
    wo_d = nc.dram_tensor("wo", [DM, DM], F32, kind="ExternalInput").ap()
    wu_d = nc.dram_tensor("wu", [2 * DM, DM], F32, kind="ExternalInput").ap()
    bq_d = nc.dram_tensor("bq", [DM], F32, kind="ExternalInput").ap()
    bk_d = nc.dram_tensor("bk", [DM], F32, kind="ExternalInput").ap()
    bv_d = nc.dram_tensor("bv", [DM], F32, kind="ExternalInput").ap()
    bo_d = nc.dram_tensor("bo", [DM], F32, kind="ExternalInput").ap()
    bu_d = nc.dram_tensor("bu", [DM], F32, kind="ExternalInput").ap()
    out_d = nc.dram_tensor("out", [N, DM], F32, kind="ExternalOutput").ap()

    def r(ap):
        return ap.bitcast(F32R)

    with tile.TileContext(nc) as tc:
        with (
            tc.tile_pool(name="const", bufs=1) as cpool,
            tc.tile_pool(name="big", bufs=1) as bpool,
            tc.tile_pool(name="work", bufs=3) as wpool,
            tc.tile_pool(name="ps", bufs=1, space="PSUM") as pp,
        ):
            # PSUM tags (8 banks total):
            #  "sc":   [128,1024] f32 x2 bufs = 4 banks (score megas; end-
            #          phase tail psum)
            #  "ctx":  [128, 512] f32 x1     = 1 bank  (row-major ctx accum,
            #          4 qblk x 2 heads x 64)
            #  "sums": [128, 512] f32 x1     = 1 bank  ([128,16] sums during
            #          attention; proj pt in phase A)
            #  "mm":   [128, 512] f32 x2     = 2 banks (proj pt / transposes)
            def sc_tile(name="scm", dtype=F32):
                return pp.tile([128, 1024], dtype, tag="sc", bufs=2, name=name)

            def mm_tile(cols=512, name="pt", tag="mm", dtype=F32):
                return pp.tile([128, cols], dtype, tag=tag, bufs=2, name=name)

            # ---- constants / weights (batched DMAs) ----
            ident = cpool.tile([128, 128], F32R, tag="ident")
            ident_f = cpool.tile([128, 128], F32, tag="identf")
            ident_bf = cpool.tile([128, 128], BF16, tag="identbf")
            make_identity(nc, ident_f[:])
            nc.vector.tensor_copy(ident[:], ident_f[:])
            nc.vector.tensor_copy(ident_bf[:], ident_f[:])
            # PE HAM warm-up during the input-DMA window; also preload the
            # ACT exp table off the critical path.
            warm = sc_tile("warm", dtype=F32R)
            for _ in range(10):
                nc.tensor.transpose(warm[0:128, 0:128], ident[:], ident[:])
            wexp = cpool.tile([1, 8], F32, tag="wexp")
            nc.scalar.activation(
                wexp[:], ident_f[0:1, 0:8], mybir.ActivationFunctionType.Exp
            )
            wq_a = cpool.tile([128, 2 * DM], F32R, tag="wqa")
            wk_a = cpool.tile([128, 2 * DM], F32R, tag="wka")
            wv_a = cpool.tile([128, 2 * DM], F32R, tag="wva")
            wo_a = cpool.tile([128, 2 * DM], F32R, tag="woa")
            wu_a = cpool.tile([128, 4 * DM], F32R, tag="wua")

            def dma_w(t_sb, t_d, eng=None):
                # startup-critical weights issue from a second DMA queue so
                # their queue/sem overheads overlap the x transfers
                (eng or nc.sync).dma_start(
                    t_sb[:].rearrange("p (c d) -> p c d", d=DM),
                    r(t_d.rearrange("(c p) d -> p c d", p=128)),
                )

            stage = cpool.tile([128, NB * DM], F32R, tag="stage")
            xs_all = stage
            x_r = r(x_d.rearrange("(t p) d -> p t d", p=128))

            def dma_x(g):
                nc.sync.dma_start(
                    xs_all[:, g * 2 * DM:(g + 1) * 2 * DM].rearrange(
                        "p (t d) -> p t d", d=DM
                    ),
                    x_r[:, g * 2:(g + 1) * 2, :],
                )

            dma_x(0)
            dma_x(1)
            dma_w(wq_a, wq_d)
            dma_w(wk_a, wk_d)
            wq_sb = [wq_a[:, c * DM:(c + 1) * DM] for c in range(2)]
            wk_sb = [wk_a[:, c * DM:(c + 1) * DM] for c in range(2)]
            wv_sb = [wv_a[:, c * DM:(c + 1) * DM] for c in range(2)]
            wo_sb = [wo_a[:, c * DM:(c + 1) * DM] for c in range(2)]
            wu_sb = [wu_a[:, c * DM:(c + 1) * DM] for c in range(4)]
            bq_a = cpool.tile([128, 2], F32, tag="bqa")
            bk_a = cpool.tile([128, 2], F32, tag="bka")
            bo_a = cpool.tile([128, 2], F32, tag="boa")
            bu_a = cpool.tile([128, 2], F32, tag="bua")
            for t_sb, t_d in ((bq_a, bq_d), (bk_a, bk_d)):
                nc.sync.dma_start(t_sb[:], t_d.rearrange("(c p) -> p c", p=128))
            dma_w(wv_a, wv_d)
            bq_c = [bq_a[:, b:b + 1] for b in range(2)]
            bk_c = [bk_a[:, b:b + 1] for b in range(2)]
            bo_c = [bo_a[:, b:b + 1] for b in range(2)]
            bu_c = [bu_a[:, b:b + 1] for b in range(2)]
            # bv broadcast tile [128, 256] (f32; only used by DVE add).
            # Broadcast on Pool so the PE stream never waits for the bv DMA.
            bv_row = cpool.tile([1, DM], F32, tag="bvrow")
            nc.sync.dma_start(bv_row[:], bv_d.rearrange("(b a) -> b a", b=1))
            bv_bc = cpool.tile([128, DM], F32, tag="bvbc")
            nc.gpsimd.partition_broadcast(bv_bc[:], bv_row[:])
            ones_col4 = cpool.tile([128, 4], F32, tag="onescol4")
            nc.gpsimd.memset(ones_col4[:], 1.0)
            # lower-triangle mask [128, 2x128] bf16 (duplicated for the two
            # heads of a merged exp tile): trimask[p, hl, j] = (j >= p)
            trimask = cpool.tile([128, 256], BF16, tag="trimask")
            nc.gpsimd.memset(trimask[:], 1.0)
            for hl in range(2):
                nc.gpsimd.affine_select(
                    trimask[:, hl * 128:(hl + 1) * 128],
                    trimask[:, hl * 128:(hl + 1) * 128],
                    pattern=[[1, 128]],
                    compare_op=mybir.AluOpType.is_ge,
                    fill=0.0,
                    base=0,
                    channel_multiplier=-1,
                )
            # ---- rest of x + remaining weights ----
            for g in range(2, 8):
                dma_x(g)
            dma_w(wo_a, wo_d)
            dma_w(wu_a, wu_d)
            # bo is consumed (bitcast) by an f32r matmul: mark the DMA f32r
            nc.sync.dma_start(
                r(bo_a[:]), r(bo_d.rearrange("(c p) -> p c", p=128))
            )
            nc.sync.dma_start(bu_a[:], bu_d.rearrange("(c p) -> p c", p=128))

            xT = [bpool.tile([128, N], F32R, tag=f"xT{c}", name=f"xT{c}") for c in range(2)]
            qT = [bpool.tile([128, N], F32R, tag=f"qT{b}", name=f"qT{b}") for b in range(2)]
            kT = [bpool.tile([128, N], F32R, tag=f"kT{b}", name=f"kT{b}") for b in range(2)]

            # x transposes for one it: 8 [128,128] transposes packed as two
            # PSUM quads + two DVE evictions.
            def emit_xt(it):
                for half in range(2):
                    tq = mm_tile(name="xtq", dtype=F32R)
                    for k in range(4):
                        ib = it * 4 + half * 2 + k // 2
                        c = k % 2
                        nc.tensor.transpose(
                            tq[:, k * 128:(k + 1) * 128],
                            xs_all[:, ib * DM + c * 128:ib * DM + (c + 1) * 128],
                            ident[:],
                        )
                    # quad holds (ib0c0, ib0c1, ib1c0, ib1c1): scatter to the
                    # two xT chunk tensors with one strided copy each
                    ib0 = it * 4 + half * 2
                    for c, eng in ((0, nc.vector), (1, nc.vector)):
                        eng.tensor_copy(
                            xT[c][:, ib0 * 128:(ib0 + 2) * 128].rearrange(
                                "p (i e) -> p i e", e=128
                            ),
                            tq[:].rearrange("p (i c e) -> p c i e", c=2, e=128)[
                                :, c
                            ],
                        )

            def emit_qk_it(blk, it, qk, split=False):
                w_sb, b_c, dstT = (
                    (wq_sb, bq_c, qT) if qk == 0 else (wk_sb, bk_c, kT)
                )
                pt = mm_tile(name="qkpt")
                for c in range(2):
                    nc.tensor.matmul(
                        pt[:],
                        w_sb[c][:, blk * 128:(blk + 1) * 128],
                        xT[c][:, it * 512:(it + 1) * 512],
                        start=(c == 0),
                        stop=(c == 1),
                    )
                if split:
                    # evict the first 128 cols separately so the first
                    # score matmul (which only needs them) starts sooner
                    nc.vector.tensor_scalar_add(
                        dstT[blk][:, it * 512:it * 512 + 128],
                        pt[:, 0:128], b_c[blk][:],
                    )
                    nc.vector.tensor_scalar_add(
                        dstT[blk][:, it * 512 + 128:(it + 1) * 512],
                        pt[:, 128:512], b_c[blk][:],
                    )
                else:
                    nc.vector.tensor_scalar_add(
                        dstT[blk][:, it * 512:(it + 1) * 512], pt[:],
                        b_c[blk][:],
                    )

            # ---- v (row-major, with ones col per head) ----
            # v_sb[jb]: [128, 4*65]; head h data at cols 65h..65h+63, ones at
            # 65h+64
            v_sb = [
                bpool.tile([128, 4 * 65], BF16, tag=f"v{jb}", name=f"v{jb}")
                for jb in range(NB)
            ]

            def emit_v(jb):
                v4 = v_sb[jb][:].rearrange("p (h e) -> p h e", e=65)
                nc.vector.tensor_copy(
                    v4[:, :, 64:65],
                    ones_col4[:].rearrange("p (h e) -> p h e", e=1),
                )
                pt = mm_tile(DM, name="vpt")
                for c in range(2):
                    nc.tensor.matmul(
                        pt[:, 0:DM],
                        xT[c][:, jb * 128:(jb + 1) * 128],
                        wv_sb[c][:],
                        start=(c == 0),
                        stop=(c == 1),
                    )
                nc.vector.tensor_tensor(
                    v4[:, :, 0:64],
                    pt[:, 0:DM].rearrange("p (h e) -> p h e", e=64),
                    bv_bc[:].rearrange("p (h e) -> p h e", e=64),
                    op=mybir.AluOpType.add,
                )

            # ---- attention state ----
            ectx = [
                bpool.tile([128, N], BF16, tag=f"ectx{b}", name=f"ectx{b}")
                for b in range(2)
            ]
            uT = [
                bpool.tile([128, N], F32R, tag=f"uT{b}", name=f"uT{b}")
                for b in range(2)
            ]
            # fused update weights: Wfused = Wo @ Wu_m folds the message
            # projection into the update GEMM (u = relu(Wu_x^T x^T +
            # Wfused^T ectx^T + (bo @ Wu_m + bu))), eliminating the whole
            # Wo pass and its PSUM round-trip. Computed on-device off the
            # critical path.
            woT_sb = cpool.tile([128, 512], F32R, tag="woT")
            wfused = cpool.tile([128, 512], BF16, tag="wfused")
            bo_wu = cpool.tile([1, DM], BF16, tag="bowu")
            ones_row = cpool.tile([1, 512], BF16, tag="onesrow")
            nc.gpsimd.memset(ones_row[:], 1.0)

            def emit_wfuse1():
                # woT blocks: tq slot (2c+c') = transpose of wo_sb[c]'s
                # c'-block
                tq = mm_tile(name="wotq", dtype=F32R)
                for c in range(2):
                    for cp in range(2):
                        nc.tensor.transpose(
                            tq[:, (2 * c + cp) * 128:(2 * c + cp + 1) * 128],
                            wo_sb[c][:, cp * 128:(cp + 1) * 128],
                            ident[:],
                        )
                nc.vector.tensor_copy(woT_sb[:], tq[:])

            def emit_wfuse2():
                pt = mm_tile(name="wfpt")
                for c in range(2):
                    for cp in range(2):
                        nc.tensor.matmul(
                            pt[:, c * 256:(c + 1) * 256],
                            woT_sb[:, (2 * c + cp) * 128:(2 * c + cp + 1) * 128],
                            wu_sb[2 + cp][:],
                            start=(cp == 0),
                            stop=(cp == 1),
                            skip_group_check=True,
                        )
                nc.vector.tensor_copy(wfused[:], pt[:])

            def emit_wfuse3():
                pt = mm_tile(DM, name="bowupt")
                for c in range(2):
                    nc.tensor.matmul(
                        pt[0:1, 0:DM],
                        bo_a[:, c:c + 1].bitcast(F32R),
                        wu_sb[2 + c][:],
                        start=(c == 0),
                        stop=(c == 1),
                    )
                nc.vector.tensor_copy(bo_wu[:], pt[0:1, 0:DM])
            ostage = stage
            out_r = r(out_d.rearrange("(t p) d -> p t d", p=128))
            scale = float(1.0 / np.sqrt(HD))

            # One attention window: head-pair hp, query i-tile it
            # (queries [512*it, 512*(it+1))), keys jb in 0..4*it+3.
            # trickle: list of callables emitted between jb steps.
            # finish_prev: previous window's deferred tail (last ctx step +
            # norms + ctx transposes), emitted after this window's pipeline
            # has ramped so the boundary never stalls ACT.
            # split_finish: end-of-kernel mode -- emit the ctx transposes in
            # two 256-col halves with the final output tail interleaved so
            # the drain chain is as short as possible.
            def attention(hp, it, trickle, finish_prev=None,
                          split_finish=None):
                qh = [qT[hp][64 * hl:64 * hl + 64, :] for hl in range(2)]
                kh = [kT[hp][64 * hl:64 * hl + 64, :] for hl in range(2)]
                cst0, cend = it * 512, (it + 1) * 512
                jb_max = 4 * it + 4
                # 4 query-chunk slots at 256-col pitch; per slot two [*,65]
                # head accumulators (ctx 64 + softmax sum col) -- offsets
                # chosen so no matmul output crosses a 2KB PSUM bank.
                # Allocated lazily at the first ctx_step so it lands AFTER
                # finish_prev's reads of the previous window's ctxp (same
                # 1-buf ring slot).
                ctx_state = {}

                def get_ctxp():
                    if "t" not in ctx_state:
                        ctx_state["t"] = pp.tile(
                            [128, 1024], F32, tag="ctx", bufs=1, name="ctxp"
                        )
                    return ctx_state["t"]

                # ctx+norm for one (already-exp'd) jb step; lags the
                # scores/exp pass by one jb so PE never queues behind exp.
                def ctx_step(jb, e):
                    ctxp = get_ctxp()
                    diag = jb // 4 == it
                    off = min(128 * (jb % 4), 256) if diag else 0
                    for lq in range(4):
                        qblk = 4 * it + lq
                        if qblk < jb:
                            continue
                        ecol = lq * 128 - off
                        for hl in range(2):
                            h = 2 * hp + hl
                            ech = e[:, hl * 512 + ecol:hl * 512 + ecol + 128]
                            # PSUM start zeroes the whole 2KB bank: emit
                            # start=True only on the first matmul touching
                            # each bank (lq 0 and 2, head 0, jb 0); the
                            # bank-wide pending-zero covers the sibling
                            # regions' first writes.
                            nc.tensor.matmul(
                                ctxp[:, lq * 256 + hl * 65:lq * 256 + hl * 65 + 65],
                                ech,
                                v_sb[jb][:, 65 * h:65 * h + 65],
                                start=(jb == 0 and hl == 0 and lq % 2 == 0),
                                stop=(jb == qblk),
                                skip_group_check=True,
                            )
                        if jb == qblk:
                            # qblk complete: normalize into ctx_sb
                            # (per-partition reciprocal of the two sums
                            # cols + scalar-multiply eviction)
                            recip = wpool.tile(
                                [128, 2], F32, tag="recip", bufs=4,
                                name="recip",
                            )
                            nc.vector.reciprocal(
                                recip[:].rearrange("p (h c) -> p h c", c=1),
                                ctxp[:, lq * 256 + 64:lq * 256 + 194].rearrange(
                                    "p (h c) -> p h c", c=65
                                )[:, :, 0:1],
                            )
                            csb = wpool.tile(
                                [128, 128], BF16, tag=f"csb{lq}", bufs=4,
                                name="csb",
                            )
                            for hl in range(2):
                                base = lq * 256 + hl * 65
                                nc.vector.tensor_scalar_mul(
                                    csb[:, hl * 64:hl * 64 + 64],
                                    ctxp[:, base:base + 64],
                                    recip[:, hl:hl + 1],
                                )
                            _CSB[lq] = csb

                def finish_half(hf, eng):
                    tq = mm_tile(name="ctqh", dtype=BF16)
                    for k in range(2):
                        lq = 2 * hf + k
                        nc.tensor.transpose(
                            tq[:, k * 128:(k + 1) * 128],
                            _CSB[lq][:], ident_bf[:],
                        )
                    eng.tensor_copy(
                        ectx[hp][:, it * 512 + hf * 256:
                                  it * 512 + (hf + 1) * 256],
                        tq[:, 0:256],
                    )
                    split_finish(hf)

                pending = None
                for jb in range(jb_max):
                    diag = jb // 4 == it
                    off = min(128 * (jb % 4), 256) if diag else 0
                    cst = cst0 + off
                    w = cend - cst
                    # leading fully-masked chunk for jb%4==3 (skip exp+ctx)
                    skip = 128 if (diag and jb % 4 == 3) else 0
                    mega = sc_tile()
                    for hl in range(2):
                        nc.tensor.matmul(
                            mega[:, hl * 512:hl * 512 + w],
                            kh[hl][:, jb * 128:(jb + 1) * 128],
                            qh[hl][:, cst:cend],
                            start=True,
                            stop=True,
                        )
                    e = wpool.tile(
                        [128, 1024], BF16, tag="e", bufs=16, name="e"
                    )
                    nc.scalar.activation(
                        e[:].rearrange("p (h c) -> p h c", h=2)[:, :, skip:w],
                        mega[:].rearrange("p (h c) -> p h c", h=2)[
                            :, :, skip:w
                        ],
                        mybir.ActivationFunctionType.Exp,
                        scale=scale,
                    )
                    if diag:
                        # zero above-diagonal inside the diagonal 128-chunk
                        # (both heads at once; chunk starts at query jb*128)
                        # via DVE multiply with the precomputed triangle
                        # mask -- much lower latency than Pool affine_select.
                        # In the end-of-kernel window's drain, DVE is the
                        # contended engine, so the last masks go to Pool.
                        doff = jb * 128 - cst
                        ev = e[:].rearrange("p (h c) -> p h c", h=2)[
                            :, :, doff:doff + 128
                        ]
                        if split_finish is not None and jb >= jb_max - 3:
                            nc.gpsimd.tensor_tensor(
                                ev, ev,
                                trimask[:].rearrange("p (h c) -> p h c", h=2),
                                op=mybir.AluOpType.mult,
                            )
                        else:
                            nc.vector.tensor_tensor(
                                ev, ev,
                                trimask[:].rearrange("p (h c) -> p h c", h=2),
                                op=mybir.AluOpType.mult,
                            )
                    if jb == 1 and finish_prev is not None:
                        finish_prev()
                    if pending is not None:
                        ctx_step(*pending)
                    if (jb == jb_max - 2 and split_finish is not None):
                        # end-of-kernel: first finish half overlaps the
                        # window's last two exp/ctx steps (csb0/csb1 exist
                        # once ctx_step(jb_max-3... qblk1) has run)
                        finish_half(0, nc.vector)
                    pending = (jb, e)
                    if trickle:
                        trickle.pop(0)()
                leftover = list(trickle)

                def finish():
                    ctx_step(*pending)
                    for fn in leftover:
                        fn()
                    if split_finish is None:
                        # transpose the 4 normalized [128,128] row-major ctx
                        # chunks back to column-major ectx (one PSUM quad +
                        # one Pool evict)
                        tq = mm_tile(name="ctq", dtype=BF16)
                        for lq in range(4):
                            nc.tensor.transpose(
                                tq[:, lq * 128:(lq + 1) * 128], _CSB[lq][:],
                                ident_bf[:],
                            )
                        nc.vector.tensor_copy(
                            ectx[hp][:, it * 512:(it + 1) * 512], tq[:]
                        )
                    else:
                        # half 0 was already emitted inside the loop
                        finish_half(1, nc.vector)

                return finish

            _CSB = {}

            def tail_u(it, blk):
                # fused update: u^T = relu(Wu_x^T x^T + Wfused^T ectx^T
                #                          + bo_wu (rank-1) + bu)
                isl = slice(it * 512, (it + 1) * 512)
                pt = mm_tile(name="wupt")
                for c in range(2):
                    nc.tensor.matmul(
                        pt[:],
                        wu_sb[c][:, blk * 128:(blk + 1) * 128],
                        xT[c][:, isl],
                        start=(c == 0),
                        stop=False,
                    )
                for c in range(2):
                    nc.tensor.matmul(
                        pt[:],
                        wfused[:, c * 256 + blk * 128:c * 256 + blk * 128 + 128],
                        ectx[c][:, isl],
                        start=False,
                        stop=False,
                    )
                nc.tensor.matmul(
                    pt[:],
                    bo_wu[0:1, blk * 128:(blk + 1) * 128],
                    ones_row[0:1, :],
                    start=False,
                    stop=True,
                )
                nc.vector.tensor_scalar(
                    uT[blk][:, isl], pt[:], bu_c[blk][:], 0.0,
                    op0=mybir.AluOpType.add, op1=mybir.AluOpType.max,
                )

            def tail_ut(it, pair):
                # transpose u^T back to row-major: 4 [128,128] transposes
                # (2 ib x 2 blk) packed into one PSUM quad, evict to ostage,
                # then DMA out the two row-blocks.
                tq = mm_tile(name="utq", dtype=F32R)
                for k in range(2):
                    ib = it * 4 + pair * 2 + k
                    for blk in range(2):
                        nc.tensor.transpose(
                            tq[:, (2 * k + blk) * 128:(2 * k + blk + 1) * 128],
                            uT[blk][:, ib * 128:(ib + 1) * 128],
                            ident[:],
                        )
                ib0 = it * 4 + pair * 2
                nc.vector.tensor_copy(
                    ostage[:, ib0 * DM:(ib0 + 2) * DM], tq[:]
                )
                nc.sync.dma_start(
                    out_r[:, ib0:ib0 + 2, :],
                    ostage[:, ib0 * DM:(ib0 + 2) * DM].rearrange(
                        "p (t d) -> p t d", d=DM
                    ),
                )

            # ---------------- schedule ----------------
            # phase A: x transposes it0 + qk0-it0 only (overlaps input DMA);
            # everything else trickles into the attention windows.
            emit_xt(0)
            emit_qk_it(0, 0, 0)
            emit_qk_it(0, 0, 1, split=True)
            emit_qk_it(1, 0, 0)
            emit_qk_it(1, 0, 1)

            def T(*fns):
                def run():
                    for f in fns:
                        f()
                return run

            qk = emit_qk_it
            # snake order: consecutive windows share a head-pair, so a new
            # q/k projection is never needed right at a window boundary.
            # Trickle lists are sized to pop before each window's last two
            # steps (late pops would drain ahead of the next window's
            # score matmuls on the in-order PE stream).
            windows = [
                # (hp, it, trickle list)
                (0, 0, [T(lambda: emit_v(0), lambda: emit_v(1)),
                        T(lambda: emit_xt(1)),
                        T(lambda: emit_v(2), lambda: emit_v(3))]),
                (1, 0, [T(lambda: qk(1, 1, 0)),
                        T(lambda: qk(1, 1, 1))]),
                (1, 1, [T(lambda: qk(0, 1, 0)),
                        T(lambda: qk(0, 1, 1)),
                        T(lambda: emit_v(4), lambda: emit_v(5)),
                        T(lambda: emit_v(6), lambda: emit_v(7)),
                        T(emit_wfuse1, lambda: emit_xt(2)),
                        ]),
                (0, 1, [T(lambda: qk(0, 2, 0)),
                        T(lambda: qk(0, 2, 1)),
                        T(emit_wfuse2),
                        T(emit_wfuse3),
                        ]),
                (0, 2, [T(lambda: qk(1, 2, 0)),
                        T(lambda: qk(1, 2, 1)),
                        T(lambda: tail_u(0, 0)),
                        T(lambda: tail_u(0, 1)),
                        T(lambda: emit_v(8)),
                        T(lambda: emit_v(9)),
                        T(lambda: emit_v(10)),
                        T(lambda: emit_v(11)),
                        T(lambda: tail_ut(0, 0), lambda: tail_ut(0, 1)),
                        T(lambda: emit_xt(3)),
                        ]),
                (1, 2, [T(lambda: qk(1, 3, 0)),
                        T(lambda: qk(1, 3, 1)),
                        T(lambda: emit_v(12)),
                        T(lambda: emit_v(13)),
                        T(lambda: emit_v(14)),
                        T(lambda: emit_v(15)),
                        ]),
                (1, 3, [T(lambda: tail_u(1, 0)),
                        T(lambda: tail_u(1, 1)),
                        T(lambda: tail_ut(1, 0), lambda: tail_ut(1, 1)),
                        T(lambda: qk(0, 3, 0)),
                        T(lambda: qk(0, 3, 1)),
                        T(lambda: tail_u(2, 0)),
                        T(lambda: tail_u(2, 1)),
                        T(lambda: tail_ut(2, 0), lambda: tail_ut(2, 1)),
                        ]),
                (0, 3, []),
            ]
            # end-game tail for it3, one 256-col half at a time so half 0
            # overlaps the last window's drain; each stage half-width to
            # shorten the final dependency chain.
            def tail3_half(hf):
                tag = "mm" if hf == 0 else "sc"

                def pt_tile(name, dtype=F32):
                    return pp.tile([128, 512], dtype, tag=tag, bufs=2,
                                   name=name)

                csl = slice(3 * 512 + hf * 256, 3 * 512 + (hf + 1) * 256)
                pt2 = pt_tile("wu3h")
                for blk in range(2):
                    for c in range(2):
                        nc.tensor.matmul(
                            pt2[:, blk * 256:(blk + 1) * 256],
                            wu_sb[c][:, blk * 128:(blk + 1) * 128],
                            xT[c][:, csl],
                            start=(c == 0), stop=False,
                            skip_group_check=True,
                        )
                    for c in range(2):
                        nc.tensor.matmul(
                            pt2[:, blk * 256:(blk + 1) * 256],
                            wfused[:, c * 256 + blk * 128:c * 256 + blk * 128 + 128],
                            ectx[c][:, csl],
                            start=False, stop=False,
                            skip_group_check=True,
                        )
                    nc.tensor.matmul(
                        pt2[:, blk * 256:(blk + 1) * 256],
                        bo_wu[0:1, blk * 128:(blk + 1) * 128],
                        ones_row[0:1, 0:256],
                        start=False, stop=True,
                        skip_group_check=True,
                    )
                for blk in range(2):
                    nc.vector.tensor_scalar(
                        uT[blk][:, csl], pt2[:, blk * 256:(blk + 1) * 256],
                        bu_c[blk][:], 0.0,
                        op0=mybir.AluOpType.add, op1=mybir.AluOpType.max,
                    )
                tq = pt_tile("ut3h", dtype=F32R)
                ib0 = 12 + 2 * hf
                for k in range(2):
                    for blk in range(2):
                        nc.tensor.transpose(
                            tq[:, (2 * k + blk) * 128:(2 * k + blk + 1) * 128],
                            uT[blk][:, (ib0 + k) * 128:(ib0 + k + 1) * 128],
                            ident[:],
                        )
                # per-block copy + DMA so the last transfer starts as early
                # as possible; the very last copy runs on the (idle) scalar
                # engine so the two copies proceed in parallel
                for k in range(2):
                    ib = ib0 + k
                    if hf == 1 and k == 1:
                        nc.scalar.copy(
                            ostage[:, ib * DM:(ib + 1) * DM],
                            tq[:, k * 256:(k + 1) * 256],
                        )
                    else:
                        nc.vector.tensor_copy(
                            ostage[:, ib * DM:(ib + 1) * DM],
                            tq[:, k * 256:(k + 1) * 256],
                        )
                    nc.sync.dma_start(
                        out_r[:, ib:ib + 1, :],
                        ostage[:, ib * DM:(ib + 1) * DM].rearrange(
                            "p (t d) -> p t d", d=DM
                        ),
                    )

            fin = None
            for i, (hp, it, trickle) in enumerate(windows):
                fin = attention(
                    hp, it, trickle, fin,
                    split_finish=tail3_half if i == len(windows) - 1 else None,
                )
            fin()

    nc.compile()
    return nc


_STATE = {}


def _get_runner():
    if "run" in _STATE:
        return _STATE["run"]
    import jax
    from concourse.bass2jax import (
        _bass_exec_p,
        install_neuronx_cc_hook,
        partition_id_tensor,
    )
    from jax.sharding import Mesh, PartitionSpec
    from jax.experimental.shard_map import shard_map

    nc = build_program()
    install_neuronx_cc_hook()
    partition_name = nc.partition_id_tensor.name if nc.partition_id_tensor else None
    in_names, out_names, out_avals, zero_outs = [], [], [], []
    for alloc in nc.m.functions[0].allocations:
        if not isinstance(alloc, mybir.MemoryLocationSet):
            continue
        name = alloc.memorylocations[0].name
        if alloc.kind == "ExternalInput":
            if name != partition_name:
                in_names.append(name)
        elif alloc.kind == "ExternalOutput":
            shape = tuple(alloc.tensor_shape)
            dtype = mybir.dt.np(alloc.dtype)
            out_names.append(name)
            out_avals.append(jax.core.ShapedArray(shape, dtype))
            zero_outs.append(np.zeros(shape, dtype))
    n_params = len(in_names)
    all_in = in_names + out_names + ([partition_name] if partition_name else [])

    def _body(*args):
        operands = list(args)
        if partition_name is not None:
            operands.append(partition_id_tensor())
        return tuple(
            _bass_exec_p.bind(
                *operands,
                out_avals=tuple(out_avals),
                in_names=tuple(all_in),
                out_names=tuple(out_names),
                lowering_input_output_aliases=(),
                sim_require_finite=True,
                sim_require_nnan=True,
                nc=nc,
            )
        )

    devices = jax.devices()[:B]
    mesh = Mesh(np.asarray(devices), ("core",))
    specs = (PartitionSpec("core"),) * (n_params + len(out_names))
    jitted = jax.jit(
        shard_map(
            _body, mesh=mesh, in_specs=specs,
            out_specs=(PartitionSpec("core"),) * len(out_names), check_rep=False,
        ),
        keep_unused=True,
    )

    def run(in_maps):
        import jax as _jax

        concat_in = [
            np.concatenate([np.asarray(m[nm]) for m in in_maps], axis=0)
            for nm in in_names
        ]
        concat_zero = [
            np.zeros((B * z.shape[0], *z.shape[1:]), z.dtype) for z in zero_outs
        ]
        outs = jitted(*concat_in, *concat_zero)
        _jax.block_until_ready(outs)
        res = []
        o = np.asarray(outs[out_names.index("out")])
        per = o.shape[0] // B
        for c in range(B):
            res.append(o[c * per:(c + 1) * per])
        return res

    _STATE["run"] = run
    return run


def make_in_maps(node_features, Wq, bq, Wk, bk, Wv, bv, Wo, bo, Wu, bu):
    in_maps = []
    for c in range(B):
        in_maps.append(
            {
                "x": np.ascontiguousarray(node_features[c], dtype=np.float32),
                "wq": np.asarray(Wq, np.float32),
                "wk": np.asarray(Wk, np.float32),
                "wv": np.asarray(Wv, np.float32),
                "wo": np.asarray(Wo, np.float32),
                "wu": np.asarray(Wu, np.float32),
                "bq": np.asarray(bq, np.float32),
                "bk": np.asarray(bk, np.float32),
                "bv": np.asarray(bv, np.float32),
                "bo": np.asarray(bo, np.float32),
                "bu": np.asarray(bu, np.float32),
            }
        )
    return in_maps


def kernel(
    node_features, causal_mask, Wq, bq, Wk, bk, Wv, bv, Wo, bo, Wu, bu
):
    """Full-input entry point: shards batch across 8 cores internally."""
    del causal_mask  # guaranteed tril(ones); mask generated on-chip
    run = _get_runner()
    in_maps = make_in_maps(node_features, Wq, bq, Wk, bk, Wv, bv, Wo, bo, Wu, bu)
    outs = run(in_maps)
    return np.stack(outs, axis=0)
